# revision 66
# baseline (speedup 1.0000x reference)
"""Trainium2 Bass kernel for AttentionMLP (nn_AttentionMLP_72997264163220).

Reference computation:
  k/q/v = x @ W{k,q,v}.T + b      (D=3800 -> D)
  scores = q @ k.T / sqrt(D); attn = softmax(scores, -1)
  attended = attn @ v; h = attended.mean(seq)
  h = sigmoid(h @ W1.T + b1); h = sigmoid(h @ W2.T + b2); out = h @ W3.T + b3

Algebraic simplifications (all host-side folds):
  1. scores = x' M x'^T with M = Wq'^T Wk' / sqrt(D) precomputed on the
     host (biases ride along in a unit feature at d=3800). q and k are
     never materialized.
  2. The mean over the sequence commutes with the attention matmul, the
     (linear) v projection AND the first MLP layer:
       h1pre = mean_i(attn) @ v @ W1^T + b1 = abar @ x' @ Z^T
     with Z = W1' @ Wv' [H, D'] precomputed on the host. So v, Wv and W1
     never appear on-device: one G = x' @ Z^T [T, H] matmul (independent
     of attention, overlapped with softmax) plus a tiny abar @ G.

Sharding: data-parallel over batch. 16 batches -> 8 cores x 2 batches
(512 tokens per core). All weights replicated, host pre-packed / cast.
Big matmuls in fp8 DoubleRow (fp32 PSUM accumulate); softmax and the
tail MLP in fp32.

Device dataflow per core (SBUF partition dim always first; D padded to
3840 = 30*128 with the bias feature at d=3800):
  x8    [128, 15, 2, 512] fp8  x'^T DR-paired (d1p, pair, ko, token)
  per d2-tile (30): t1[d2t] = M^T x'  (15 DRSW matmuls, N=512; m8 is
  host-packed in the DoubleRowSwInterleave layout so LDWEIGHTS reads
  contiguously and hides fully under the matmul stream)
  score MMs interleaved 2 chunks behind: ps[2b+it] += t1_pair^T x8_pair
  G = x'-pairs @ Z^T (60 DR matmuls) interleaved into the early chunks,
  where the PE would otherwise wait on the m8/x8 DMA race
  softmax without max-subtraction (scores are O(1)): pexp = exp(ps/SC)
  bf16, r = 1/rowsum; abar = pexp^T r via matmul (attn never normalized)
  h1T[128h,4ot,2b] = sigmoid(G^T abar / (ZSC*S)); bf16 W2 layer, fp32
  final layer; biases via unit rows.

Latency hiding: ~32 warmup matmuls on zeroed SBUF run during the
startup DMA shadow so the PE HAM clock-gate opens (1.2 -> 2.4 GHz)
before real work; dummy Exp/Sigmoid activations preload the 1.3us ACT
function tables off the critical path. DMA uses both HWDGE rings (SP:
m8 stream + zt/w2/w3 behind it; ACT: x8 in 4 grouped descriptors) —
ring-push cost (~0.6us/descriptor) was the startup bottleneck.
"""

import sys
import types

import numpy as np

if "/opt/trn_rl_repo" not in sys.path:
    sys.path.insert(0, "/opt/trn_rl_repo")


# ---------------------------------------------------------------------------
# NTFF profile hook shim (antenv.axon_hooks is absent in this image). Needed
# only when profiling (trace=True); harmless otherwise.
# ---------------------------------------------------------------------------
def _install_ntff_hook():
    try:
        import antenv  # noqa: F401

        if "antenv.axon_hooks" in sys.modules:
            return
        hooks_mod = types.ModuleType("antenv.axon_hooks")
        hooks_mod._hook = None

        def set_axon_ntff_profile_hook(h):
            hooks_mod._hook = h

        def get_axon_ntff_profile_hook():
            return hooks_mod._hook

        hooks_mod.set_axon_ntff_profile_hook = set_axon_ntff_profile_hook
        hooks_mod.get_axon_ntff_profile_hook = get_axon_ntff_profile_hook
        sys.modules["antenv.axon_hooks"] = hooks_mod
        import antenv as _a

        _a.axon_hooks = hooks_mod
        from trn_agent_boot.trn_boot import _ntff_profile_via_ctypes

        set_axon_ntff_profile_hook(
            _ntff_profile_via_ctypes("/opt/axon/libaxon_pjrt.so")
        )
    except Exception:
        pass


_install_ntff_hook()


def _install_verbose_cc_hook():
    """Wrap the PJRT->python compile callback so real tracebacks surface
    instead of an opaque 'CallFunctionObjArgs' error."""
    try:
        import traceback

        from concourse import bass2jax

        bass2jax.install_neuronx_cc_hook()
        import libneuronxla

        if getattr(libneuronxla, "_ant_verbose_wrap", False):
            return
        orig = libneuronxla.neuronx_cc

        def wrapped(*a, **k):
            try:
                return orig(*a, **k)
            except BaseException:
                traceback.print_exc()
                sys.stderr.flush()
                raise

        libneuronxla.neuronx_cc = wrapped
        libneuronxla._ant_verbose_wrap = True
        bass2jax.install_neuronx_cc_hook = lambda: None
    except Exception:
        pass


import bass_rust
import ml_dtypes
import concourse.bass as bass
import concourse.tile as tile
from concourse import mybir
from concourse.bass_utils import run_bass_kernel_spmd
from concourse.vector_clock import ScopedClock

BF16 = ml_dtypes.bfloat16

N_CORES = 8
B = 16  # batches total
S = 256  # seq len
D = 3800  # feature dim
H = 512  # hidden
C = 10  # classes

BLOC = B // N_CORES  # batches per core = 2
T = BLOC * S  # tokens per core = 512
DP = 3840  # D padded (+1 bias feature, up to 30*128)
KC = DP // 128  # 30 contraction chunks
ET = DP // 128  # 30 e-tiles of 128
PAIRS = KC // 2  # 15 DoubleRow chunk pairs
F32 = mybir.dt.float32
BF = mybir.dt.bfloat16
F8 = mybir.dt.float8e4
F8NP = mybir.dt.np(F8)  # ml_dtypes.float8_e4m3
# fp8 scale factors: weights are ~U(+-1/sqrt(3800)) which lands in e4m3's
# subnormal range, so weights are scaled up and the product scales are
# folded back out downstream (softmax scale / sigmoid scale).
SC_SCALE = 4096.0  # on M = Wq^T Wk / sqrt(D); scores' = 4096 * scores
ZSC = 32.0  # on Z = W1' @ Wv'; h1pre' = ZSC * h1pre


class SplitDrainTileContext(tile.TileContext):
    """This walrus build rejects >1 sync-wait on the tail Drain; split the
    global-clock waits across a chain of single-wait drain instructions."""

    MAXW = 1

    def _drain_and_barrier(self, tick_clock, wait_clock):
        nc = self.nc
        drain_inst = nc.sync.drain()
        wait_clock.add_sem_waits(
            drain_inst.ins, ScopedClock({None: tick_clock.global_clock})
        )
        si = drain_inst.ins.sync_info
        if si is not None and si.on_wait and len(si.on_wait) > self.MAXW:
            waits = list(si.on_wait)
            si.on_wait = waits[: self.MAXW]
            rest = waits[self.MAXW :]
            for i in range(0, len(rest), self.MAXW):
                extra = nc.sync.drain()
                extra.ins.sync_info = bass_rust.SyncInfo(
                    on_wait=rest[i : i + self.MAXW], on_update=[]
                )
        nc.all_engine_barrier()
        assert self.sems is not None
        popped = nc._tile_sem_poison_stack.pop()
        assert popped is self._sem_poison
        nc.clear_and_free_semaphores(list(self.sems.allocated().values()))
        nc.all_engine_barrier()


def _fix_excess_waits(nc, aux_sem, maxw=1):
    """Walrus in this image rejects instructions with more than ~1 sync
    wait. Compute-engine instructions: hoist extra waits onto same-engine
    no-ops inserted just before (sequencers execute in order). DMACopy:
    its waits live in the DGE queue descriptor, so an SP-side chain waits
    on all the original conditions, bumps `aux_sem`, and the descriptor
    waits on aux_sem alone."""
    aux_count = 0
    for f in nc.m.functions:
        for bb in f.blocks:
            insts = bb.instructions
            if not any(
                i.sync_info and i.sync_info.on_wait
                and len(i.sync_info.on_wait) > maxw
                for i in insts
            ):
                continue
            out = []
            for ins in insts:
                si = ins.sync_info
                nw = len(si.on_wait) if si and si.on_wait else 0
                if nw > maxw:
                    waits = list(si.on_wait)
                    if isinstance(ins, mybir.InstDMACopy):
                        for j, w in enumerate(waits):
                            nop = mybir.InstNoOp(name=f"{ins.name}-dw{j}")
                            nop.engine = mybir.EngineType.SP
                            nop.sync_info = bass_rust.SyncInfo(
                                on_wait=[w], on_update=[]
                            )
                            out.append(nop)
                        aux_count += 1
                        inc = mybir.InstNoOp(name=f"{ins.name}-dinc")
                        inc.engine = mybir.EngineType.SP
                        inc.sync_info = bass_rust.SyncInfo(
                            on_wait=[],
                            on_update=[
                                bass_rust.SyncUpdate(
                                    sync_type="semaphore",
                                    id=aux_sem.num,
                                    ant_name=aux_sem.name,
                                    update_mode="sem-add-imm",
                                    update_value=1,
                                    update_reg=None,
                                )
                            ],
                        )
                        out.append(inc)
                        si.on_wait = [
                            bass_rust.SyncWait(
                                sync_type="semaphore",
                                id=aux_sem.num,
                                ant_name=aux_sem.name,
                                wait_mode="sem-ge-imm",
                                wait_value=aux_count,
                                wait_reg=None,
                            )
                        ]
                    else:
                        keep = waits[-maxw:]
                        rest = waits[:-maxw]
                        for j, w in enumerate(rest):
                            nop = mybir.InstNoOp(name=f"{ins.name}-xw{j}")
                            nop.engine = ins.engine
                            nop.sync_info = bass_rust.SyncInfo(
                                on_wait=[w], on_update=[]
                            )
                            out.append(nop)
                        si.on_wait = keep
                out.append(ins)
            bb.instructions = out
    if aux_count:
        # reset aux sem at the very end so a re-executed NEFF starts clean
        f = nc.m.functions[0]
        bb = list(f.blocks)[-1]
        rst = mybir.InstNoOp(name="auxwait-reset")
        rst.engine = mybir.EngineType.SP
        rst.sync_info = bass_rust.SyncInfo(
            on_wait=[],
            on_update=[
                bass_rust.SyncUpdate(
                    sync_type="semaphore",
                    id=aux_sem.num,
                    ant_name=aux_sem.name,
                    update_mode="sem-sub-imm",
                    update_value=aux_count,
                    update_reg=None,
                )
            ],
        )
        il = bb.instructions
        il.append(rst)
        bb.instructions = il


def build_kernel() -> bass.Bass:
    nc = bass.Bass()

    x_d = nc.declare_dram_parameter("x8", [128, PAIRS, 2, T], F8, isOutput=False)
    m8_d = nc.declare_dram_parameter("m8", [ET, 128, PAIRS, 2, 128], F8,
                                     isOutput=False)
    zt_d = nc.declare_dram_parameter("zt", [128, PAIRS, 2, H], F8, isOutput=False)
    w2_d = nc.declare_dram_parameter("w2", [128, 5, H], BF, isOutput=False)
    w3_d = nc.declare_dram_parameter("w3", [128, 5, C], F32, isOutput=False)
    out_d = nc.declare_dram_parameter("outT", [C, BLOC], F32, isOutput=True)

    aux_sem = nc.alloc_semaphore("auxwait")
    with SplitDrainTileContext(nc) as tc:
        with tc.tile_pool(name="persist", bufs=1) as persist:
            _emit(nc, tc, persist, x_d, m8_d, zt_d, w2_d, w3_d, out_d)
    _fix_excess_waits(nc, aux_sem)
    return nc


def _emit(nc, tc, persist, x_d, m8_d, zt_d, w2_d, w3_d, out_d):
    # ------------------ persistent tiles ------------------
    # x8 as one tile, DMAd in 4 grouped descriptors (subtile deps let the
    # early pair matmuls start as soon as their group lands).
    x8t = persist.tile([128, PAIRS, 2, T], F8, name="x8t")
    ab4 = persist.tile([128, 4], BF, name="ab4")
    # t1 = (M8^T x8): fp8, [d2 within tile, d2-tile, token]
    t1_sb = persist.tile([128, KC, T], F8)
    zt_t = persist.tile([128, PAIRS, 2, H], F8)
    g_sb = persist.tile([128, 4, H], BF)

    # MLP weights: tiles up-front, DMAs issued a few iterations into
    # phase 1 so they overlap compute instead of the critical startup
    mlpw = tc.alloc_tile_pool(name="mlpw", bufs=1)
    w2_t = mlpw.tile([128, 5, H], BF)
    w3_t = mlpw.tile([128, 5, C], F32)

    # ---- phase 1a: t1 = M^T x  (scores = x M x^T = t1^T x, M = Wq^T Wk) ----
    # Score matmuls (phase 1b) are interleaved into the same loop with a
    # 2-chunk lag so the PE never waits on the DVE psum->sbuf cast.
    # m8 is host-packed in the SwInterleave layout so its LDWEIGHTS reads
    # contiguously (FWL-speed) instead of the 2x-slow DR interleave read.
    DR = mybir.MatmulPerfMode.DoubleRow
    DRSW = mybir.MatmulPerfMode.DoubleRowSwInterleave
    with tc.tile_pool(name="psum_sc", bufs=1, space="PSUM") as psum_sc:
        # both it-halves of a batch's scores share one PSUM bank so a
        # single wide Exp covers them (shortens the serial ACT chain)
        ps = [
            psum_sc.tile([128, 2, S], F32, name=f"scores{b}", tag=f"scores{b}")
            for b in range(BLOC)
        ]

        def emit_scores_pair(p):
            for b in range(BLOC):
                for it in range(2):
                    i0 = b * S + it * 128
                    nc.tensor.matmul(
                        ps[b][:, it, :],
                        t1_sb[:, 2 * p : 2 * p + 2, i0 : i0 + 128],
                        x8t[:, p, :, b * S : (b + 1) * S],
                        start=(p == 0), stop=(p == PAIRS - 1),
                        perf_mode=DR,
                        skip_group_check=True,
                    )

        # G-matmul interleave schedule: G = x' @ Z^T is independent of the
        # attention, so its 60 matmuls hide inside the phase-1 stream —
        # biased toward the EARLY chunks, where the m8 DMA stream is still
        # catching up behind the x8 block and the PE would otherwise gap at
        # every chunk boundary. Token-chunks (0,1) accumulate over
        # d2t=0..14, (2,3) over 15..29 (2 PSUM banks at a time; ps 4 +
        # pt 2 + pg 2 = 8 banks).
        gsched = {d2t: [] for d2t in range(ET)}
        for d2t in range(3, 18):
            gsched[d2t] = [(0, d2t - 3), (1, d2t - 3)]
        gq = [(tcn, p) for p in range(PAIRS) for tcn in (2, 3)]
        for d2t in range(18, ET):
            take = 3 if d2t < 24 else 2
            gsched[d2t], gq = gq[:take], gq[take:]
        assert not gq

        with (
            tc.tile_pool(name="mpool", bufs=1) as mpool,
            tc.tile_pool(name="psum_kq", bufs=1, space="PSUM") as psum_kq,
            tc.tile_pool(name="psum_g", bufs=1, space="PSUM") as psum_g,
        ):
            # HAM warmup: ~20 dummy matmuls on uninitialized SBUF run while
            # the first DMAs stream in, so the PE clock-gate opens (4/8 ->
            # 8/8 after ~3.4us of activity) before the first real matmul.
            warm_w = persist.tile([128, S], F8, name="warm_w")
            nc.vector.memset(warm_w[:], 0.0)
            pt0 = psum_kq.tile([128, T], F32, tag="pt", bufs=3, name="pt0")
            for _ in range(32):
                nc.tensor.matmul(pt0[:, 0:S], warm_w[:, 0:128], warm_w[:],
                                 start=True, stop=True)

            pg_h = {}

            def g_mm(tcn, p):
                if p == 0:
                    pg_h[tcn] = psum_g.tile(
                        [128, H], F32, tag="pg", bufs=2, name=f"pg{tcn}"
                    )
                nc.tensor.matmul(
                    pg_h[tcn][:],
                    x8t[:, p, :, tcn * 128 : (tcn + 1) * 128],
                    zt_t[:, p],
                    start=(p == 0), stop=(p == PAIRS - 1),
                    perf_mode=DR,
                    skip_group_check=True,
                )
                if p == PAIRS - 1:
                    nc.vector.tensor_copy(g_sb[:, tcn, :], pg_h[tcn][:])

            for d2t in range(ET):
                m_t = mpool.tile([128, PAIRS, 2, 128], F8, tag="m8", bufs=3)
                if d2t == 0:
                    # Two HWDGE rings in parallel: the SP ring carries only
                    # m8 (2 descriptors for chunk 0, then 1/chunk) while the
                    # ACT ring carries x8 (4 grouped descriptors) + zt +
                    # w2/w3. The ring-push cost (~0.6us/descriptor) was the
                    # early-phase bottleneck on a single ring.
                    nc.sync.dma_start(m_t[:, 0:8], m8_d[0, :, 0:8])
                    nc.sync.dma_start(m_t[:, 8:PAIRS], m8_d[0, :, 8:PAIRS])
                    for lo, hi in ((0, 5), (5, 10), (10, PAIRS)):
                        nc.scalar.dma_start(x8t[:, lo:hi], x_d[:, lo:hi])
                    # dummy Exp (after the x8 ring-pushes in the Scalar
                    # queue): pulls the 1.3us Exp ACT-table load off the
                    # softmax critical path into the startup DMA shadow.
                    # (The table RAM holds one function; Sigmoid gets the
                    # same treatment right after the softmax Exps.)
                    act_w = persist.tile([128, 1], F32, name="act_w")
                    nc.vector.memset(act_w[:], 0.0)
                    act_o = persist.tile([128, 1], F32, name="act_o")
                    nc.scalar.activation(
                        act_o[:], act_w[:],
                        mybir.ActivationFunctionType.Exp,
                    )
                else:
                    nc.sync.dma_start(m_t[:], m8_d[d2t])
                # zt/w2/w3 ride the SP ring BEHIND the first m8 chunks so
                # the early chunk stream never loses the DMA-engine
                # bandwidth race to them (zt isn't needed before chunk 3's
                # G matmuls, w2/w3 not until the MLP tail).
                if d2t == 2:
                    nc.sync.dma_start(zt_t[:, 0:8], zt_d[:, 0:8])
                if d2t == 3:
                    nc.sync.dma_start(zt_t[:, 8:PAIRS], zt_d[:, 8:PAIRS])
                if d2t == 5:
                    nc.sync.dma_start(w2_t[:], w2_d[:])
                    nc.sync.dma_start(w3_t[:], w3_d[:])

                pt = psum_kq.tile([128, T], F32, tag="pt", bufs=3)
                for p in range(PAIRS):
                    nc.tensor.matmul(
                        pt[:], m_t[:, p], x8t[:, p],
                        start=(p == 0), stop=(p == PAIRS - 1),
                        perf_mode=DRSW,
                    )
                nc.vector.tensor_copy(t1_sb[:, d2t, :], pt[:])
                if d2t >= 3 and d2t % 2 == 1:
                    emit_scores_pair((d2t - 3) // 2)
                for tcn, p in gsched[d2t]:
                    g_mm(tcn, p)
            emit_scores_pair(PAIRS - 1)

        # ---- phase 2: softmax + abar (PE idles only for the ACT chain) ----
        with (
            tc.tile_pool(name="smx", bufs=1) as smx,
            tc.tile_pool(name="psum_ab", bufs=1, space="PSUM") as psum_ab,
        ):
            pexps = []
            rs = []
            sm = None
            for b in range(BLOC):
                for it in range(2):
                    pexp = smx.tile([128, S], BF, tag=f"pexp{2*b+it}")
                    # no max-subtraction: scores are O(1) by construction.
                    # Rowsums run on the (idle) DVE instead of the ACT
                    # accumulator so the 4-Exp chain on ACT is shorter.
                    nc.scalar.activation(
                        pexp[:], ps[b][:, it, :],
                        mybir.ActivationFunctionType.Exp,
                        scale=1.0 / SC_SCALE,
                    )
                    sm = smx.tile([128, 1], F32, tag="sm", bufs=2)
                    nc.vector.reduce_sum(
                        out=sm[:], in_=pexp[:], axis=mybir.AxisListType.X
                    )
                    r = smx.tile([128, 1], BF, tag="r", bufs=4)
                    # r = 1/rowsum; the 1/S mean is folded into the sigmoid
                    # scale of the first MLP layer. bf16 r: its ~0.2% rel
                    # error perturbs each row's attention mass uniformly and
                    # averages out across the 256-row mean-pool.
                    with nc.allow_low_precision(reason="bf16 softmax norm"):
                        nc.vector.reciprocal(r[:], sm[:])
                    pexps.append(pexp)
                    rs.append(r)

            # dummy Sigmoid: its 1.3us table load runs on ACT while the PE
            # chews the abar/h1 matmuls, so the real h1 sigmoid is instant.
            # Reads the last Exp's rowsum so the Tile scheduler cannot
            # hoist it before the softmax (it would evict the Exp table).
            nc.scalar.activation(
                act_o[:], sm[:], mybir.ActivationFunctionType.Sigmoid
            )

            # abar[j] = sum_i pexp[i, j] * r[i]  (bf16 matmul, N=1)
            pab = psum_ab.tile([128, 4], F32, name="pab")
            for b in range(BLOC):
                for it in range(2):
                    for jc in range(2):
                        nc.tensor.matmul(
                            pab[:, 2 * b + jc : 2 * b + jc + 1],
                            pexps[2 * b + it][:, jc * 128 : (jc + 1) * 128],
                            rs[2 * b + it][:],
                            start=(it == 0), stop=(it == 1),
                            skip_group_check=True,
                        )
            nc.vector.tensor_copy(ab4[:], pab[:])

    # ------------------ phase 3: MLP (fp32) ------------------
    # All 4 output-chunks of each layer accumulate into one PSUM tile so a
    # single sigmoid ACT covers the layer: minimizes dependency hops in
    # this purely latency-bound tail.
    with (
        tc.tile_pool(name="mlph", bufs=1) as mlph,
        tc.tile_pool(name="psum_m", bufs=1, space="PSUM") as psum_m,
    ):
        h1T = mlph.tile([128, 5, BLOC], BF)
        nc.vector.memset(h1T[:, 4, :], 0.0)
        nc.vector.memset(h1T[0:1, 4, :], 1.0)
        pm1 = psum_m.tile([128, 4, BLOC], F32, tag="pm1")
        for ot in range(4):
            for b in range(BLOC):
                for tt in (2 * b, 2 * b + 1):
                    nc.tensor.matmul(
                        pm1[:, ot, b : b + 1],
                        g_sb[:, tt, ot * 128 : (ot + 1) * 128],
                        ab4[:, tt : tt + 1],
                        start=(tt == 2 * b), stop=(tt == 2 * b + 1),
                        skip_group_check=True,
                    )
        nc.scalar.activation(
            h1T[:, 0:4, :], pm1[:], mybir.ActivationFunctionType.Sigmoid,
            scale=1.0 / (ZSC * S),
        )

        h2T = mlph.tile([128, 5, BLOC], F32)
        nc.vector.memset(h2T[:, 4, :], 0.0)
        nc.vector.memset(h2T[0:1, 4, :], 1.0)
        pm2 = psum_m.tile([128, 4, BLOC], F32, tag="pm2")
        for ot in range(4):
            for oc in range(5):
                nc.tensor.matmul(
                    pm2[:, ot, :],
                    w2_t[:, oc, ot * 128 : (ot + 1) * 128],
                    h1T[:, oc, :],
                    start=(oc == 0), stop=(oc == 4),
                    skip_group_check=True,
                )
        nc.scalar.activation(
            h2T[:, 0:4, :], pm2[:], mybir.ActivationFunctionType.Sigmoid
        )

        pm3 = psum_m.tile([C, BLOC], F32, tag="pm3")
        for oc in range(5):
            nc.tensor.matmul(
                pm3[:],
                w3_t[:, oc, :],
                h2T[:, oc, :],
                start=(oc == 0), stop=(oc == 4),
            )
        out_sb = mlph.tile([C, BLOC], F32)
        nc.vector.tensor_copy(out_sb[:], pm3[:])
        nc.sync.dma_start(out_d[:], out_sb[:])
    mlpw.release()


# ---------------------------------------------------------------------------
# Host-side packing
# ---------------------------------------------------------------------------
def _pack_m8(Wq, bq, Wk, bk):
    """M = Wq'^T Wk' / sqrt(D), where W' carries its bias in column d=3800.
    scores = x' M x'^T reproduces q @ k.T / sqrt(D) exactly (the unit bias
    feature of x' supplies the bias cross terms). Scaled by SC_SCALE for
    e4m3 range, packed in the DoubleRowSwInterleave stationary layout:
    per (d2t, d1p, pair) the 256 weights are stored as
    [A_127, B_127, A_126, B_126, ..., A_0, B_0] where
    A_j = M[(2p+0)*128+d1p, d2t*128+127-j], B_j likewise for ko=1."""
    Wqp = np.zeros((D, DP), dtype=np.float32)
    Wqp[:, :D] = Wq
    Wqp[:, D] = bq
    Wkp = np.zeros((D, DP), dtype=np.float32)
    Wkp[:, :D] = Wk
    Wkp[:, D] = bk
    M = (Wqp.T @ Wkp) * np.float32(SC_SCALE / np.sqrt(np.float64(D)))
    A5 = M.reshape(PAIRS, 2, 128, ET, 128)  # [pr, ko, d1p, d2t, d2p]
    Asw = A5[:, :, :, :, ::-1].transpose(3, 2, 0, 4, 1)  # [d2t,d1p,pr,j,ko]
    return np.ascontiguousarray(Asw, dtype=F8NP).reshape(ET, 128, PAIRS, 2, 128)


def _pack_zt(W1, b1, Wv, bv):
    """Z = W1' @ Wv' [H, DP]: the v projection and the first MLP layer
    fused. Wv' carries bv in column d=3800 plus a unit at [3800, 3800] so
    x's bias feature flows through; b1 is added into Z's column 3800.
    Packed for the DR moving operand: zt[d1p, pair, ko, o] =
    ZSC * Z[o, (2*pair+ko)*128 + d1p] -> [128, PAIRS, 2, H]."""
    Wvp = np.zeros((DP, DP), dtype=np.float32)
    Wvp[:D, :D] = Wv
    Wvp[:D, D] = bv
    Wvp[D, D] = 1.0
    W1p = np.zeros((H, DP), dtype=np.float32)
    W1p[:, :D] = W1
    W1p[:, D] = b1
    Z = (W1p @ Wvp) * np.float32(ZSC)  # [H, DP]
    A = Z.T.reshape(PAIRS, 2, 128, H).transpose(2, 0, 1, 3)
    return np.ascontiguousarray(A, dtype=F8NP)


def _pack_x8(xc):
    """xc [BLOC, S, D] -> [128, PAIRS, 2, T] e4m3, bias row d=3800 = 1."""
    xt = np.zeros((DP, T), dtype=np.float32)
    xt[:D, :] = xc.reshape(T, D).T
    xt[D, :] = 1.0
    A = xt.reshape(PAIRS, 2, 128, T).transpose(2, 0, 1, 3)
    return np.ascontiguousarray(A, dtype=F8NP)


def _pack_w2(W2, b2):
    A = np.zeros((128, 5, H), dtype=np.float32)
    A[:, :4, :] = W2.T.reshape(4, 128, H).transpose(1, 0, 2)
    A[0, 4, :] = b2
    return np.ascontiguousarray(A, dtype=BF16)


def _pack_w3(W3, b3):
    A = np.zeros((128, 5, C), dtype=np.float32)
    A[:, :4, :] = W3.T.reshape(4, 128, C).transpose(1, 0, 2)
    A[0, 4, :] = b3
    return np.ascontiguousarray(A)


_NC_CACHE = {}


def _get_nc():
    if "nc" not in _NC_CACHE:
        _NC_CACHE["nc"] = build_kernel()
    return _NC_CACHE["nc"]


def kernel(x, Wk, bk, Wq, bq, Wv, bv, W1, b1, W2, b2, W3, b3, _trace=False):
    x = np.asarray(x, dtype=np.float32)

    m8_p = _pack_m8(
        np.asarray(Wq, np.float32), np.asarray(bq, np.float32),
        np.asarray(Wk, np.float32), np.asarray(bk, np.float32),
    )
    zt_p = _pack_zt(
        np.asarray(W1, np.float32), np.asarray(b1, np.float32),
        np.asarray(Wv, np.float32), np.asarray(bv, np.float32),
    )
    w2_p = _pack_w2(np.asarray(W2, np.float32), np.asarray(b2, np.float32))
    w3_p = _pack_w3(np.asarray(W3, np.float32), np.asarray(b3, np.float32))

    in_maps = []
    for c in range(N_CORES):
        xc = x[c * BLOC : (c + 1) * BLOC]
        in_maps.append(
            {
                "x8": _pack_x8(xc),
                "m8": m8_p,
                "zt": zt_p,
                "w2": w2_p,
                "w3": w3_p,
            }
        )

    nc = _get_nc()
    _install_verbose_cc_hook()
    res = run_bass_kernel_spmd(nc, in_maps, list(range(N_CORES)), trace=_trace)
    out = np.zeros((B, C), dtype=np.float32)
    for c in range(N_CORES):
        out[c * BLOC : (c + 1) * BLOC] = res.results[c]["outT"].T
    if _trace:
        return out, res
    return out



# revision 68
# speedup vs baseline: 1.0052x; 1.0052x over previous
"""Trainium2 Bass kernel for AttentionMLP (nn_AttentionMLP_72997264163220).

Reference computation:
  k/q/v = x @ W{k,q,v}.T + b      (D=3800 -> D)
  scores = q @ k.T / sqrt(D); attn = softmax(scores, -1)
  attended = attn @ v; h = attended.mean(seq)
  h = sigmoid(h @ W1.T + b1); h = sigmoid(h @ W2.T + b2); out = h @ W3.T + b3

Algebraic simplifications (all host-side folds):
  1. scores = x' M x'^T with M = Wq'^T Wk' / sqrt(D) precomputed on the
     host (biases ride along in a unit feature at d=3800). q and k are
     never materialized.
  2. The mean over the sequence commutes with the attention matmul, the
     (linear) v projection AND the first MLP layer:
       h1pre = mean_i(attn) @ v @ W1^T + b1 = abar @ x' @ Z^T
     with Z = W1' @ Wv' [H, D'] precomputed on the host. So v, Wv and W1
     never appear on-device: one G = x' @ Z^T [T, H] matmul (independent
     of attention, overlapped with softmax) plus a tiny abar @ G.

Sharding: data-parallel over batch. 16 batches -> 8 cores x 2 batches
(512 tokens per core). All weights replicated, host pre-packed / cast.
Big matmuls in fp8 DoubleRow (fp32 PSUM accumulate); softmax and the
tail MLP in fp32.

Device dataflow per core (SBUF partition dim always first; D padded to
3840 = 30*128 with the bias feature at d=3800):
  x8    [128, 15, 2, 512] fp8  x'^T DR-paired (d1p, pair, ko, token)
  per d2-tile (30): t1[d2t] = M^T x'  (15 DRSW matmuls, N=512; m8 is
  host-packed in the DoubleRowSwInterleave layout so LDWEIGHTS reads
  contiguously and hides fully under the matmul stream)
  score MMs interleaved 2 chunks behind: ps[2b+it] += t1_pair^T x8_pair
  G = x'-pairs @ Z^T (60 DR matmuls) interleaved into the early chunks,
  where the PE would otherwise wait on the m8/x8 DMA race
  softmax without max-subtraction (scores are O(1)): pexp = exp(ps/SC)
  bf16, r = 1/rowsum; abar = pexp^T r via matmul (attn never normalized)
  h1T[128h,4ot,2b] = sigmoid(G^T abar / (ZSC*S)); bf16 W2 layer, fp32
  final layer; biases via unit rows.

Latency hiding: ~32 warmup matmuls on zeroed SBUF run during the
startup DMA shadow so the PE HAM clock-gate opens (1.2 -> 2.4 GHz)
before real work; dummy Exp/Sigmoid activations preload the 1.3us ACT
function tables off the critical path. DMA uses both HWDGE rings (SP:
m8 stream + zt/w2/w3 behind it; ACT: x8 in 4 grouped descriptors) —
ring-push cost (~0.6us/descriptor) was the startup bottleneck.
"""

import sys
import types

import numpy as np

if "/opt/trn_rl_repo" not in sys.path:
    sys.path.insert(0, "/opt/trn_rl_repo")


# ---------------------------------------------------------------------------
# NTFF profile hook shim (antenv.axon_hooks is absent in this image). Needed
# only when profiling (trace=True); harmless otherwise.
# ---------------------------------------------------------------------------
def _install_ntff_hook():
    try:
        import antenv  # noqa: F401

        if "antenv.axon_hooks" in sys.modules:
            return
        hooks_mod = types.ModuleType("antenv.axon_hooks")
        hooks_mod._hook = None

        def set_axon_ntff_profile_hook(h):
            hooks_mod._hook = h

        def get_axon_ntff_profile_hook():
            return hooks_mod._hook

        hooks_mod.set_axon_ntff_profile_hook = set_axon_ntff_profile_hook
        hooks_mod.get_axon_ntff_profile_hook = get_axon_ntff_profile_hook
        sys.modules["antenv.axon_hooks"] = hooks_mod
        import antenv as _a

        _a.axon_hooks = hooks_mod
        from trn_agent_boot.trn_boot import _ntff_profile_via_ctypes

        set_axon_ntff_profile_hook(
            _ntff_profile_via_ctypes("/opt/axon/libaxon_pjrt.so")
        )
    except Exception:
        pass


_install_ntff_hook()


def _install_verbose_cc_hook():
    """Wrap the PJRT->python compile callback so real tracebacks surface
    instead of an opaque 'CallFunctionObjArgs' error."""
    try:
        import traceback

        from concourse import bass2jax

        bass2jax.install_neuronx_cc_hook()
        import libneuronxla

        if getattr(libneuronxla, "_ant_verbose_wrap", False):
            return
        orig = libneuronxla.neuronx_cc

        def wrapped(*a, **k):
            try:
                return orig(*a, **k)
            except BaseException:
                traceback.print_exc()
                sys.stderr.flush()
                raise

        libneuronxla.neuronx_cc = wrapped
        libneuronxla._ant_verbose_wrap = True
        bass2jax.install_neuronx_cc_hook = lambda: None
    except Exception:
        pass


import bass_rust
import ml_dtypes
import concourse.bass as bass
import concourse.tile as tile
from concourse import mybir
from concourse.bass_utils import run_bass_kernel_spmd
from concourse.vector_clock import ScopedClock

BF16 = ml_dtypes.bfloat16

N_CORES = 8
B = 16  # batches total
S = 256  # seq len
D = 3800  # feature dim
H = 512  # hidden
C = 10  # classes

BLOC = B // N_CORES  # batches per core = 2
T = BLOC * S  # tokens per core = 512
DP = 3840  # D padded (+1 bias feature, up to 30*128)
KC = DP // 128  # 30 contraction chunks
ET = DP // 128  # 30 e-tiles of 128
PAIRS = KC // 2  # 15 DoubleRow chunk pairs
F32 = mybir.dt.float32
BF = mybir.dt.bfloat16
F8 = mybir.dt.float8e4
F8NP = mybir.dt.np(F8)  # ml_dtypes.float8_e4m3
# fp8 scale factors: weights are ~U(+-1/sqrt(3800)) which lands in e4m3's
# subnormal range, so weights are scaled up and the product scales are
# folded back out downstream (softmax scale / sigmoid scale).
SC_SCALE = 4096.0  # on M = Wq^T Wk / sqrt(D); scores' = 4096 * scores
ZSC = 32.0  # on Z = W1' @ Wv'; h1pre' = ZSC * h1pre


class SplitDrainTileContext(tile.TileContext):
    """This walrus build rejects >1 sync-wait on the tail Drain; split the
    global-clock waits across a chain of single-wait drain instructions."""

    MAXW = 1

    def _drain_and_barrier(self, tick_clock, wait_clock):
        nc = self.nc
        drain_inst = nc.sync.drain()
        wait_clock.add_sem_waits(
            drain_inst.ins, ScopedClock({None: tick_clock.global_clock})
        )
        si = drain_inst.ins.sync_info
        if si is not None and si.on_wait and len(si.on_wait) > self.MAXW:
            waits = list(si.on_wait)
            si.on_wait = waits[: self.MAXW]
            rest = waits[self.MAXW :]
            for i in range(0, len(rest), self.MAXW):
                extra = nc.sync.drain()
                extra.ins.sync_info = bass_rust.SyncInfo(
                    on_wait=rest[i : i + self.MAXW], on_update=[]
                )
        nc.all_engine_barrier()
        assert self.sems is not None
        popped = nc._tile_sem_poison_stack.pop()
        assert popped is self._sem_poison
        nc.clear_and_free_semaphores(list(self.sems.allocated().values()))
        nc.all_engine_barrier()


def _fix_excess_waits(nc, aux_sem, maxw=1):
    """Walrus in this image rejects instructions with more than ~1 sync
    wait. Compute-engine instructions: hoist extra waits onto same-engine
    no-ops inserted just before (sequencers execute in order). DMACopy:
    its waits live in the DGE queue descriptor, so an SP-side chain waits
    on all the original conditions, bumps `aux_sem`, and the descriptor
    waits on aux_sem alone."""
    aux_count = 0
    for f in nc.m.functions:
        for bb in f.blocks:
            insts = bb.instructions
            if not any(
                i.sync_info and i.sync_info.on_wait
                and len(i.sync_info.on_wait) > maxw
                for i in insts
            ):
                continue
            out = []
            for ins in insts:
                si = ins.sync_info
                nw = len(si.on_wait) if si and si.on_wait else 0
                if nw > maxw:
                    waits = list(si.on_wait)
                    if isinstance(ins, mybir.InstDMACopy):
                        for j, w in enumerate(waits):
                            nop = mybir.InstNoOp(name=f"{ins.name}-dw{j}")
                            nop.engine = mybir.EngineType.SP
                            nop.sync_info = bass_rust.SyncInfo(
                                on_wait=[w], on_update=[]
                            )
                            out.append(nop)
                        aux_count += 1
                        inc = mybir.InstNoOp(name=f"{ins.name}-dinc")
                        inc.engine = mybir.EngineType.SP
                        inc.sync_info = bass_rust.SyncInfo(
                            on_wait=[],
                            on_update=[
                                bass_rust.SyncUpdate(
                                    sync_type="semaphore",
                                    id=aux_sem.num,
                                    ant_name=aux_sem.name,
                                    update_mode="sem-add-imm",
                                    update_value=1,
                                    update_reg=None,
                                )
                            ],
                        )
                        out.append(inc)
                        si.on_wait = [
                            bass_rust.SyncWait(
                                sync_type="semaphore",
                                id=aux_sem.num,
                                ant_name=aux_sem.name,
                                wait_mode="sem-ge-imm",
                                wait_value=aux_count,
                                wait_reg=None,
                            )
                        ]
                    else:
                        keep = waits[-maxw:]
                        rest = waits[:-maxw]
                        for j, w in enumerate(rest):
                            nop = mybir.InstNoOp(name=f"{ins.name}-xw{j}")
                            nop.engine = ins.engine
                            nop.sync_info = bass_rust.SyncInfo(
                                on_wait=[w], on_update=[]
                            )
                            out.append(nop)
                        si.on_wait = keep
                out.append(ins)
            bb.instructions = out
    if aux_count:
        # reset aux sem at the very end so a re-executed NEFF starts clean
        f = nc.m.functions[0]
        bb = list(f.blocks)[-1]
        rst = mybir.InstNoOp(name="auxwait-reset")
        rst.engine = mybir.EngineType.SP
        rst.sync_info = bass_rust.SyncInfo(
            on_wait=[],
            on_update=[
                bass_rust.SyncUpdate(
                    sync_type="semaphore",
                    id=aux_sem.num,
                    ant_name=aux_sem.name,
                    update_mode="sem-sub-imm",
                    update_value=aux_count,
                    update_reg=None,
                )
            ],
        )
        il = bb.instructions
        il.append(rst)
        bb.instructions = il


def build_kernel() -> bass.Bass:
    nc = bass.Bass()

    x_d = nc.declare_dram_parameter("x8", [128, PAIRS, 2, T], F8, isOutput=False)
    m8_d = nc.declare_dram_parameter("m8", [ET, 128, PAIRS, 2, 128], F8,
                                     isOutput=False)
    zt_d = nc.declare_dram_parameter("zt", [128, PAIRS, 2, H], F8, isOutput=False)
    w2_d = nc.declare_dram_parameter("w2", [128, 5, H], BF, isOutput=False)
    w3_d = nc.declare_dram_parameter("w3", [128, 5, C], F32, isOutput=False)
    out_d = nc.declare_dram_parameter("outT", [C, BLOC], F32, isOutput=True)

    aux_sem = nc.alloc_semaphore("auxwait")
    with SplitDrainTileContext(nc) as tc:
        with tc.tile_pool(name="persist", bufs=1) as persist:
            _emit(nc, tc, persist, x_d, m8_d, zt_d, w2_d, w3_d, out_d)
    _fix_excess_waits(nc, aux_sem)
    return nc


def _emit(nc, tc, persist, x_d, m8_d, zt_d, w2_d, w3_d, out_d):
    # ------------------ persistent tiles ------------------
    # x8 as one tile, DMAd in 4 grouped descriptors (subtile deps let the
    # early pair matmuls start as soon as their group lands).
    x8t = persist.tile([128, PAIRS, 2, T], F8, name="x8t")
    ab4 = persist.tile([128, 4], BF, name="ab4")
    # t1 = (M8^T x8): fp8, [d2 within tile, d2-tile, token]
    t1_sb = persist.tile([128, KC, T], F8)
    zt_t = persist.tile([128, PAIRS, 2, H], F8)
    g_sb = persist.tile([128, 4, H], BF)

    # MLP weights: tiles up-front, DMAs issued a few iterations into
    # phase 1 so they overlap compute instead of the critical startup
    mlpw = tc.alloc_tile_pool(name="mlpw", bufs=1)
    w2_t = mlpw.tile([128, 5, H], BF)
    w3_t = mlpw.tile([128, 5, C], F32)

    # ---- phase 1a: t1 = M^T x  (scores = x M x^T = t1^T x, M = Wq^T Wk) ----
    # Score matmuls (phase 1b) are interleaved into the same loop with a
    # 2-chunk lag so the PE never waits on the DVE psum->sbuf cast.
    # m8 is host-packed in the SwInterleave layout so its LDWEIGHTS reads
    # contiguously (FWL-speed) instead of the 2x-slow DR interleave read.
    DR = mybir.MatmulPerfMode.DoubleRow
    DRSW = mybir.MatmulPerfMode.DoubleRowSwInterleave
    with tc.tile_pool(name="psum_sc", bufs=1, space="PSUM") as psum_sc:
        # both it-halves of a batch's scores share one PSUM bank so a
        # single wide Exp covers them (shortens the serial ACT chain)
        ps = [
            psum_sc.tile([128, 2, S], F32, name=f"scores{b}", tag=f"scores{b}")
            for b in range(BLOC)
        ]

        def emit_scores_pair(p):
            for b in range(BLOC):
                for it in range(2):
                    i0 = b * S + it * 128
                    nc.tensor.matmul(
                        ps[b][:, it, :],
                        t1_sb[:, 2 * p : 2 * p + 2, i0 : i0 + 128],
                        x8t[:, p, :, b * S : (b + 1) * S],
                        start=(p == 0), stop=(p == PAIRS - 1),
                        perf_mode=DR,
                        skip_group_check=True,
                    )

        # G-matmul interleave schedule: G = x' @ Z^T is independent of the
        # attention, so its 60 matmuls hide inside the phase-1 stream —
        # biased toward the EARLY chunks, where the m8 DMA stream is still
        # catching up behind the x8 block and the PE would otherwise gap at
        # every chunk boundary. Token-chunks (0,1) accumulate over
        # d2t=0..14, (2,3) over 15..29 (2 PSUM banks at a time; ps 4 +
        # pt 2 + pg 2 = 8 banks).
        gsched = {d2t: [] for d2t in range(ET)}
        for d2t in range(3, 18):
            gsched[d2t] = [(0, d2t - 3), (1, d2t - 3)]
        gq = [(tcn, p) for p in range(PAIRS) for tcn in (2, 3)]
        for d2t in range(18, ET):
            take = 3 if d2t < 24 else 2
            gsched[d2t], gq = gq[:take], gq[take:]
        assert not gq

        with (
            tc.tile_pool(name="mpool", bufs=1) as mpool,
            tc.tile_pool(name="psum_kq", bufs=1, space="PSUM") as psum_kq,
            tc.tile_pool(name="psum_g", bufs=1, space="PSUM") as psum_g,
        ):
            # HAM warmup: ~20 dummy matmuls on uninitialized SBUF run while
            # the first DMAs stream in, so the PE clock-gate opens (4/8 ->
            # 8/8 after ~3.4us of activity) before the first real matmul.
            warm_w = persist.tile([128, S], F8, name="warm_w")
            nc.vector.memset(warm_w[:], 0.0)
            pt0 = psum_kq.tile([128, T], F32, tag="pt", bufs=3, name="pt0")
            for _ in range(32):
                nc.tensor.matmul(pt0[:, 0:S], warm_w[:, 0:128], warm_w[:],
                                 start=True, stop=True)

            pg_h = {}

            def g_mm(tcn, p):
                if p == 0:
                    pg_h[tcn] = psum_g.tile(
                        [128, H], F32, tag="pg", bufs=2, name=f"pg{tcn}"
                    )
                nc.tensor.matmul(
                    pg_h[tcn][:],
                    x8t[:, p, :, tcn * 128 : (tcn + 1) * 128],
                    zt_t[:, p],
                    start=(p == 0), stop=(p == PAIRS - 1),
                    perf_mode=DR,
                    skip_group_check=True,
                )
                if p == PAIRS - 1:
                    nc.vector.tensor_copy(g_sb[:, tcn, :], pg_h[tcn][:])

            for d2t in range(ET):
                m_t = mpool.tile([128, PAIRS, 2, 128], F8, tag="m8", bufs=3)
                if d2t == 0:
                    # Two HWDGE rings in parallel: the SP ring carries only
                    # m8 (2 descriptors for chunk 0, then 1/chunk) while the
                    # ACT ring carries x8 (4 grouped descriptors) + zt +
                    # w2/w3. The ring-push cost (~0.6us/descriptor) was the
                    # early-phase bottleneck on a single ring.
                    nc.sync.dma_start(m_t[:, 0:8], m8_d[0, :, 0:8])
                    nc.sync.dma_start(m_t[:, 8:PAIRS], m8_d[0, :, 8:PAIRS])
                    for lo, hi in ((0, 5), (5, 10), (10, PAIRS)):
                        nc.scalar.dma_start(x8t[:, lo:hi], x_d[:, lo:hi])
                    # dummy Exp (after the x8 ring-pushes in the Scalar
                    # queue): pulls the 1.3us Exp ACT-table load off the
                    # softmax critical path into the startup DMA shadow.
                    # (The table RAM holds one function; Sigmoid gets the
                    # same treatment right after the softmax Exps.)
                    act_w = persist.tile([128, 1], F32, name="act_w")
                    nc.vector.memset(act_w[:], 0.0)
                    act_o = persist.tile([128, 1], F32, name="act_o")
                    nc.scalar.activation(
                        act_o[:], act_w[:],
                        mybir.ActivationFunctionType.Exp,
                    )
                else:
                    nc.sync.dma_start(m_t[:], m8_d[d2t])
                # zt/w2/w3 ride the SP ring BEHIND the first m8 chunks so
                # the early chunk stream never loses the DMA-engine
                # bandwidth race to them (zt isn't needed before chunk 3's
                # G matmuls, w2/w3 not until the MLP tail).
                if d2t == 2:
                    nc.sync.dma_start(zt_t[:, 0:8], zt_d[:, 0:8])
                if d2t == 3:
                    nc.sync.dma_start(zt_t[:, 8:PAIRS], zt_d[:, 8:PAIRS])
                if d2t == 5:
                    nc.sync.dma_start(w2_t[:], w2_d[:])
                    nc.sync.dma_start(w3_t[:], w3_d[:])

                pt = psum_kq.tile([128, T], F32, tag="pt", bufs=3)
                for p in range(PAIRS):
                    nc.tensor.matmul(
                        pt[:], m_t[:, p], x8t[:, p],
                        start=(p == 0), stop=(p == PAIRS - 1),
                        perf_mode=DRSW,
                    )
                nc.vector.tensor_copy(t1_sb[:, d2t, :], pt[:])
                if d2t >= 3 and d2t % 2 == 1:
                    emit_scores_pair((d2t - 3) // 2)
                for tcn, p in gsched[d2t]:
                    g_mm(tcn, p)
            emit_scores_pair(PAIRS - 1)

        # ---- phase 2: softmax + abar (PE idles only for the ACT chain) ----
        with (
            tc.tile_pool(name="smx", bufs=1) as smx,
            tc.tile_pool(name="psum_ab", bufs=1, space="PSUM") as psum_ab,
        ):
            pexps = []
            rs = []
            sm = None
            for b in range(BLOC):
                for it in range(2):
                    pexp = smx.tile([128, S], BF, tag=f"pexp{2*b+it}")
                    # no max-subtraction: scores are O(1) by construction.
                    # Rowsums run on the (idle) DVE instead of the ACT
                    # accumulator so the 4-Exp chain on ACT is shorter.
                    nc.scalar.activation(
                        pexp[:], ps[b][:, it, :],
                        mybir.ActivationFunctionType.Exp,
                        scale=1.0 / SC_SCALE,
                    )
                    sm = smx.tile([128, 1], F32, tag="sm", bufs=2)
                    nc.vector.reduce_sum(
                        out=sm[:], in_=pexp[:], axis=mybir.AxisListType.X
                    )
                    r = smx.tile([128, 1], BF, tag="r", bufs=4)
                    # r = 1/rowsum; the 1/S mean is folded into the sigmoid
                    # scale of the first MLP layer. bf16 r: its ~0.2% rel
                    # error perturbs each row's attention mass uniformly and
                    # averages out across the 256-row mean-pool.
                    with nc.allow_low_precision(reason="bf16 softmax norm"):
                        nc.vector.reciprocal(r[:], sm[:])
                    pexps.append(pexp)
                    rs.append(r)

            # dummy Sigmoid: its 1.3us table load runs on ACT while the PE
            # chews the abar/h1 matmuls, so the real h1 sigmoid is instant.
            # Reads the last Exp's rowsum so the Tile scheduler cannot
            # hoist it before the softmax (it would evict the Exp table).
            nc.scalar.activation(
                act_o[:], sm[:], mybir.ActivationFunctionType.Sigmoid
            )

            # abar[j] = sum_i pexp[i, j] * r[i]  (bf16 matmul, N=1)
            pab = psum_ab.tile([128, 4], F32, name="pab")
            for b in range(BLOC):
                for it in range(2):
                    for jc in range(2):
                        nc.tensor.matmul(
                            pab[:, 2 * b + jc : 2 * b + jc + 1],
                            pexps[2 * b + it][:, jc * 128 : (jc + 1) * 128],
                            rs[2 * b + it][:],
                            start=(it == 0), stop=(it == 1),
                            skip_group_check=True,
                        )
            nc.vector.tensor_copy(ab4[:], pab[:])

    # ------------------ phase 3: MLP (fp32) ------------------
    # All 4 output-chunks of each layer accumulate into one PSUM tile so a
    # single sigmoid ACT covers the layer: minimizes dependency hops in
    # this purely latency-bound tail.
    with (
        tc.tile_pool(name="mlph", bufs=1) as mlph,
        tc.tile_pool(name="psum_m", bufs=1, space="PSUM") as psum_m,
    ):
        h1T = mlph.tile([128, 5, BLOC], BF)
        nc.vector.memset(h1T[:, 4, :], 0.0)
        nc.vector.memset(h1T[0:1, 4, :], 1.0)
        pm1 = psum_m.tile([128, 4, BLOC], F32, tag="pm1")
        for ot in range(4):
            for b in range(BLOC):
                for tt in (2 * b, 2 * b + 1):
                    nc.tensor.matmul(
                        pm1[:, ot, b : b + 1],
                        g_sb[:, tt, ot * 128 : (ot + 1) * 128],
                        ab4[:, tt : tt + 1],
                        start=(tt == 2 * b), stop=(tt == 2 * b + 1),
                        skip_group_check=True,
                    )
        nc.scalar.activation(
            h1T[:, 0:4, :], pm1[:], mybir.ActivationFunctionType.Sigmoid,
            scale=1.0 / (ZSC * S),
        )

        h2T = mlph.tile([128, 5, BLOC], F32)
        nc.vector.memset(h2T[:, 4, :], 0.0)
        nc.vector.memset(h2T[0:1, 4, :], 1.0)
        pm2 = psum_m.tile([128, 4, BLOC], F32, tag="pm2")
        for ot in range(4):
            for oc in range(5):
                nc.tensor.matmul(
                    pm2[:, ot, :],
                    w2_t[:, oc, ot * 128 : (ot + 1) * 128],
                    h1T[:, oc, :],
                    start=(oc == 0), stop=(oc == 4),
                    skip_group_check=True,
                )
        nc.scalar.activation(
            h2T[:, 0:4, :], pm2[:], mybir.ActivationFunctionType.Sigmoid
        )

        pm3 = psum_m.tile([C, BLOC], F32, tag="pm3")
        for oc in range(5):
            nc.tensor.matmul(
                pm3[:],
                w3_t[:, oc, :],
                h2T[:, oc, :],
                start=(oc == 0), stop=(oc == 4),
            )
        out_sb = mlph.tile([C, BLOC], F32)
        nc.vector.tensor_copy(out_sb[:], pm3[:])
        nc.sync.dma_start(out_d[:], out_sb[:])
    mlpw.release()


# ---------------------------------------------------------------------------
# Host-side packing
# ---------------------------------------------------------------------------
def _pack_m8(Wq, bq, Wk, bk):
    """M = Wq'^T Wk' / sqrt(D), where W' carries its bias in column d=3800.
    scores = x' M x'^T reproduces q @ k.T / sqrt(D) exactly (the unit bias
    feature of x' supplies the bias cross terms). Scaled by SC_SCALE for
    e4m3 range, packed in the DoubleRowSwInterleave stationary layout:
    per (d2t, d1p, pair) the 256 weights are stored as
    [A_127, B_127, A_126, B_126, ..., A_0, B_0] where
    A_j = M[(2p+0)*128+d1p, d2t*128+127-j], B_j likewise for ko=1."""
    Wqp = np.zeros((D, DP), dtype=np.float32)
    Wqp[:, :D] = Wq
    Wqp[:, D] = bq
    Wkp = np.zeros((D, DP), dtype=np.float32)
    Wkp[:, :D] = Wk
    Wkp[:, D] = bk
    M = (Wqp.T @ Wkp) * np.float32(SC_SCALE / np.sqrt(np.float64(D)))
    A5 = M.reshape(PAIRS, 2, 128, ET, 128)  # [pr, ko, d1p, d2t, d2p]
    Asw = A5[:, :, :, :, ::-1].transpose(3, 2, 0, 4, 1)  # [d2t,d1p,pr,j,ko]
    return np.ascontiguousarray(Asw, dtype=F8NP).reshape(ET, 128, PAIRS, 2, 128)


def _pack_zt(W1, b1, Wv, bv):
    """Z = W1' @ Wv' [H, DP]: the v projection and the first MLP layer
    fused. Wv' carries bv in column d=3800 plus a unit at [3800, 3800] so
    x's bias feature flows through; b1 is added into Z's column 3800.
    Packed for the DR moving operand: zt[d1p, pair, ko, o] =
    ZSC * Z[o, (2*pair+ko)*128 + d1p] -> [128, PAIRS, 2, H]."""
    Wvp = np.zeros((DP, DP), dtype=np.float32)
    Wvp[:D, :D] = Wv
    Wvp[:D, D] = bv
    Wvp[D, D] = 1.0
    W1p = np.zeros((H, DP), dtype=np.float32)
    W1p[:, :D] = W1
    W1p[:, D] = b1
    Z = (W1p @ Wvp) * np.float32(ZSC)  # [H, DP]
    A = Z.T.reshape(PAIRS, 2, 128, H).transpose(2, 0, 1, 3)
    return np.ascontiguousarray(A, dtype=F8NP)


def _pack_x8(xc):
    """xc [BLOC, S, D] -> [128, PAIRS, 2, T] e4m3, bias row d=3800 = 1."""
    xt = np.zeros((DP, T), dtype=np.float32)
    xt[:D, :] = xc.reshape(T, D).T
    xt[D, :] = 1.0
    A = xt.reshape(PAIRS, 2, 128, T).transpose(2, 0, 1, 3)
    return np.ascontiguousarray(A, dtype=F8NP)


def _pack_w2(W2, b2):
    A = np.zeros((128, 5, H), dtype=np.float32)
    A[:, :4, :] = W2.T.reshape(4, 128, H).transpose(1, 0, 2)
    A[0, 4, :] = b2
    return np.ascontiguousarray(A, dtype=BF16)


def _pack_w3(W3, b3):
    A = np.zeros((128, 5, C), dtype=np.float32)
    A[:, :4, :] = W3.T.reshape(4, 128, C).transpose(1, 0, 2)
    A[0, 4, :] = b3
    return np.ascontiguousarray(A)


_NC_CACHE = {}


def _get_nc():
    if "nc" not in _NC_CACHE:
        _NC_CACHE["nc"] = build_kernel()
    return _NC_CACHE["nc"]


def kernel(x, Wk, bk, Wq, bq, Wv, bv, W1, b1, W2, b2, W3, b3, _trace=False):
    x = np.asarray(x, dtype=np.float32)

    m8_p = _pack_m8(
        np.asarray(Wq, np.float32), np.asarray(bq, np.float32),
        np.asarray(Wk, np.float32), np.asarray(bk, np.float32),
    )
    zt_p = _pack_zt(
        np.asarray(W1, np.float32), np.asarray(b1, np.float32),
        np.asarray(Wv, np.float32), np.asarray(bv, np.float32),
    )
    w2_p = _pack_w2(np.asarray(W2, np.float32), np.asarray(b2, np.float32))
    w3_p = _pack_w3(np.asarray(W3, np.float32), np.asarray(b3, np.float32))

    in_maps = []
    for c in range(N_CORES):
        xc = x[c * BLOC : (c + 1) * BLOC]
        in_maps.append(
            {
                "x8": _pack_x8(xc),
                "m8": m8_p,
                "zt": zt_p,
                "w2": w2_p,
                "w3": w3_p,
            }
        )

    nc = _get_nc()
    _install_verbose_cc_hook()
    res = run_bass_kernel_spmd(nc, in_maps, list(range(N_CORES)), trace=_trace)
    out = np.zeros((B, C), dtype=np.float32)
    for c in range(N_CORES):
        out[c * BLOC : (c + 1) * BLOC] = res.results[c]["outT"].T
    if _trace:
        return out, res
    return out

